# revision 21
# baseline (speedup 1.0000x reference)
"""Trainium2 Bass kernel for nn_KeypointLoss (S=3, B=8, K=11, C=23, H=W=256).

Data-parallel over batch B=8 across 8 NeuronCores: core b computes the three
losses (heatmap / label / mask) for batch element b; host assembles [B,S].

v3: memory-regime optimized.
 - Bulk tensors (hm_pred, gt heatmaps, masks) staged in DRAM as bf16 (host
   cast): halves HBM traffic vs fp32 and enables DVE 2x tensor_tensor mode.
   (Device-side fp8->bf16 cast-DMA measured ~2x slower per SDMA engine, so
   the cast is done on host instead.)
 - msk_pred stays fp32 (BCE log terms near p->1 are precision-critical).
 - Heat loss: DVE mult (+mask broadcast), DVE sub, ACT Square+accum.
 - Rowmax: DVE TT-max halving tree (2x bf16) + final 1x reduce on [P,K,64].
 - Label loss batched across all 3 stacks ([33 = S*K] partition rows):
   argmax via reversed-iota max trick (picks first match, always in range),
   peak row refetch + 7-channel label-pred gather via two indirect DMAs from
   row-contiguous DRAM tables, BCE on [33,7] fp32.
 - gt+mask and pred loads issued on separate HWDGE queues (sync / scalar).
 - All small constants packed into one [128, 800] fp32 param -> one DMA.
"""

import numpy as np

S = 3
B = 8
K = 11
C = 23
P = 128
F = 512  # 256*256 = 128*512 plane layout
RG = K + 1  # gt+mask chunks per partition row in gtm
NACC = 12  # 3 heat + 3 ln1mp + 3 m*dd + 3 label cols

# column offsets inside the packed const param
_C_IDENT = 0          # [128, 128]
_C_IOTP = 128         # [33, 128] value 128-p
_C_IOTF = 256         # [33, 512] value 512-f
_C_C1 = 768           # [33, 1]  1536*(s+1) + k
_C_C2 = 769           # [33, 1]  65536*(s+1) + 512
_C_LAB = 770          # [33, 8]  labels tiled (7 used)
_C_WM = 778           # [12, 9]
_C_SMASK = 787        # [33, 3] stack row masks
_CW = 800

_CACHE = {}


def _build_nc():
    import concourse.bass as bass
    import concourse.bacc as bacc
    import concourse.mybir as mybir
    import concourse.tile as tile

    dt = mybir.dt
    f32, i32, bf16 = dt.float32, dt.int32, dt.bfloat16
    Alu = mybir.AluOpType
    Act = mybir.ActivationFunctionType
    AX = mybir.AxisListType.X

    nc = bacc.Bacc("TRN2", target_bir_lowering=False, debug=False)
    gtm = nc.declare_dram_parameter("gtm", [S, P, RG * F], bf16, isOutput=False)
    pr = nc.declare_dram_parameter("pr", [S, P, K * F], bf16, isOutput=False)
    mp = nc.declare_dram_parameter("mp", [S, P, F], f32, isOutput=False)
    lab8 = nc.declare_dram_parameter("lab8", [S * P * F, 8], f32, isOutput=False)
    ckp = nc.declare_dram_parameter("cpk", [128, _CW], f32, isOutput=False)
    out = nc.declare_dram_parameter("out", [1, 16], f32, isOutput=True)

    # row-contiguous table for the gt-row refetch: row (s,p,r) r in [0,12)
    gtrows = gtm[:].rearrange("s p (r f) -> (s p r) f", f=F)

    with tile.TileContext(nc) as tc:
        with (
            tc.tile_pool(name="const", bufs=1) as cst,
            tc.tile_pool(name="accp", bufs=1) as accp,
            tc.tile_pool(name="gtp", bufs=3) as gtp,
            tc.tile_pool(name="prp", bufs=3) as prp,
            tc.tile_pool(name="mpp", bufs=3) as mpp,
            tc.tile_pool(name="rmx", bufs=2) as rmx,
            tc.tile_pool(name="lns", bufs=2) as lns,
            tc.tile_pool(name="sm", bufs=2) as sm,
            tc.tile_pool(name="ps", bufs=2, space="PSUM") as ps,
        ):
            # ---------------- constants: one DMA ----------------
            ck = cst.tile([128, _CW], f32)
            nc.sync.dma_start(out=ck[:], in_=ckp[:])
            ident = ck[:, _C_IDENT:_C_IDENT + 128]
            iotp_t = ck[0:33, _C_IOTP:_C_IOTP + 128]
            iotf_t = ck[0:33, _C_IOTF:_C_IOTF + F]
            c1t = ck[0:33, _C_C1:_C_C1 + 1]
            c2t = ck[0:33, _C_C2:_C_C2 + 1]
            lab33 = ck[0:33, _C_LAB:_C_LAB + 7]
            Wm = ck[0:NACC, _C_WM:_C_WM + 9]
            ones = cst.tile([128, 1], f32)
            nc.vector.memset(ones[:], 1.0)

            acc = accp.tile([128, NACC], f32)
            nc.vector.memset(acc[:], 0.0)
            rmall = accp.tile([128, 33], f32)

            # warm the Ln/Square ACT table set during the DMA ramp
            lnwarm = accp.tile([1, 1], f32)
            nc.scalar.activation(out=lnwarm[:], in_=ones[0:1, 0:1], func=Act.Ln)

            # ---------------- per-stack main loop ----------------
            tail = {}
            import contextlib
            for s in range(S):
                gtt = gtp.tile([P, RG, F], bf16, tag="gt")
                if s == 0:
                    # first load feeds the first compute: use both HWDGE rings
                    nc.sync.dma_start(out=gtt[:, 0:6, :], in_=gtm[s][:, 0:6 * F])
                    nc.scalar.dma_start(out=gtt[:, 6:RG, :],
                                        in_=gtm[s][:, 6 * F:RG * F])
                else:
                    nc.sync.dma_start(out=gtt[:], in_=gtm[s])
                prt = prp.tile([P, K, F], bf16, tag="pr")
                nc.scalar.dma_start(out=prt[:], in_=pr[s])
                mpt = mpp.tile([P, F], f32, tag="mp")
                nc.scalar.dma_start(out=mpt[:], in_=mp[s])

                gt = gtt[:, 0:K, :]
                mskb = gtt[:, K:RG, :]   # [P,1,F]
                mskf = gtt[:, K, :]      # [P,F]

                # last stack: pull the whole tree+label chain forward so the
                # two indirect-gather latencies hide under mult/sub
                prio = tc.high_priority() if s == S - 1 else contextlib.nullcontext()
                with prio:
                    # ---- rowmax tree: label pipeline depends only on gt
                    t256 = rmx.tile([P, K, 256], bf16, tag="t256")
                    nc.vector.tensor_tensor(out=t256[:], in0=gt[:, :, 0:256],
                                            in1=gt[:, :, 256:512], op=Alu.max)
                    t128 = rmx.tile([P, K, 128], bf16, tag="t128")
                    nc.vector.tensor_tensor(out=t128[:], in0=t256[:, :, 0:128],
                                            in1=t256[:, :, 128:256], op=Alu.max)
                    t64 = rmx.tile([P, K, 64], bf16, tag="t64")
                    nc.vector.tensor_tensor(out=t64[:], in0=t128[:, :, 0:64],
                                            in1=t128[:, :, 64:128], op=Alu.max)
                    nc.vector.tensor_reduce(out=rmall[:, s * K:(s + 1) * K],
                                            in_=t64[:], axis=AX, op=Alu.max)

                if s == S - 1:
                    ctx2 = tc.high_priority()
                    ctx2.__enter__()
                    # ---- label head: argmax row + refetch gather, hoisted so
                    # the indirect-DMA latency hides under mult/sub below
                    pt = ps.tile([33, 128], f32, tag="pt")
                    nc.tensor.transpose(out=pt[:], in_=rmall[:], identity=ident)
                    rmT = sm.tile([33, 128], f32, tag="rmT")
                    nc.vector.tensor_copy(rmT[:], pt[:])
                    Mx = sm.tile([33, 1], f32, tag="Mx")
                    nc.vector.tensor_reduce(out=Mx[:], in_=rmT[:], axis=AX,
                                            op=Alu.max)
                    oh = sm.tile([33, 128], f32, tag="oh")
                    nc.vector.tensor_scalar(out=oh[:], in0=rmT[:],
                                            scalar1=Mx[:, 0:1], scalar2=None,
                                            op0=Alu.is_equal)
                    scrp = sm.tile([33, 128], f32, tag="scrp")
                    nc.vector.tensor_tensor(out=scrp[:], in0=oh[:], in1=iotp_t,
                                            op=Alu.mult)
                    pmr = sm.tile([33, 1], f32, tag="pmr")  # 128 - p*
                    nc.vector.tensor_reduce(out=pmr[:], in_=scrp[:], axis=AX,
                                            op=Alu.max)
                    idxg = sm.tile([33, 1], f32, tag="idxg")
                    nc.vector.scalar_tensor_tensor(out=idxg[:], in0=pmr[:],
                                                   scalar=-float(RG), in1=c1t,
                                                   op0=Alu.mult, op1=Alu.add)
                    idxgi = sm.tile([33, 1], i32, tag="idxgi")
                    nc.vector.tensor_copy(idxgi[:], idxg[:])
                    grow = sm.tile([33, F], bf16, tag="grow")
                    nc.gpsimd.indirect_dma_start(
                        out=grow[:], out_offset=None, in_=gtrows,
                        in_offset=bass.IndirectOffsetOnAxis(ap=idxgi[:, 0:1],
                                                            axis=0))
                    tail.update(pmr=pmr, Mx=Mx, grow=grow)
                    ctx2.__exit__(None, None, None)

                # ---- heatmap loss: sum_{k,pix} (pred*mask - gt)^2
                nc.vector.tensor_tensor(out=prt[:], in0=prt[:],
                                        in1=mskb.to_broadcast([P, K, F]),
                                        op=Alu.mult)
                nc.vector.tensor_tensor(out=prt[:], in0=prt[:], in1=gt,
                                        op=Alu.subtract)
                nc.scalar.activation(out=prt[:], in_=prt[:], func=Act.Square,
                                     accum_out=acc[:, s:s + 1])

                if s == S - 1:
                    ctx3 = tc.high_priority()
                    ctx3.__enter__()
                    # ---- label mid: column select + label-pred gather
                    pmr, Mx, grow = tail["pmr"], tail["Mx"], tail["grow"]
                    wsel = sm.tile([33, F], f32, tag="wsel")
                    nc.vector.tensor_scalar(out=wsel[:], in0=grow[:],
                                            scalar1=Mx[:, 0:1], scalar2=None,
                                            op0=Alu.is_equal)
                    scrf = sm.tile([33, F], f32, tag="scrf")
                    nc.vector.tensor_tensor(out=scrf[:], in0=wsel[:],
                                            in1=iotf_t, op=Alu.mult)
                    fmr = sm.tile([33, 1], f32, tag="fmr")  # 512 - f*
                    nc.vector.tensor_reduce(out=fmr[:], in_=scrf[:], axis=AX,
                                            op=Alu.max)
                    t1 = sm.tile([33, 1], f32, tag="t1")
                    nc.vector.scalar_tensor_tensor(out=t1[:], in0=pmr[:],
                                                   scalar=-512.0, in1=c2t,
                                                   op0=Alu.mult, op1=Alu.add)
                    gx = sm.tile([33, 1], f32, tag="gx")
                    nc.vector.tensor_tensor(out=gx[:], in0=t1[:], in1=fmr[:],
                                            op=Alu.subtract)
                    gxi = sm.tile([33, 1], i32, tag="gxi")
                    nc.vector.tensor_copy(gxi[:], gx[:])
                    G8 = sm.tile([33, 8], f32, tag="G8")
                    nc.gpsimd.indirect_dma_start(
                        out=G8[:], out_offset=None, in_=lab8[:],
                        in_offset=bass.IndirectOffsetOnAxis(ap=gxi[:, 0:1],
                                                            axis=0))
                    valid = sm.tile([33, 1], f32, tag="valid")
                    nc.vector.tensor_scalar(out=valid[:], in0=Mx[:],
                                            scalar1=1.0, scalar2=None,
                                            op0=Alu.is_equal)
                    tail.update(G8=G8, valid=valid)
                    ctx3.__exit__(None, None, None)

                # ---- mask loss: BCE(msk_pred, mask) summed
                ln1 = lns.tile([P, F], bf16, tag="ln1")
                lnp = lns.tile([P, F], bf16, tag="lnp")
                nc.scalar.activation(out=ln1[:], in_=mpt[:], func=Act.Ln,
                                     bias=1.0, scale=-1.0,
                                     accum_out=acc[:, 3 + s:4 + s])
                nc.scalar.activation(out=lnp[:], in_=mpt[:], func=Act.Ln)
                dd = lns.tile([P, F], bf16, tag="dd")
                nc.vector.tensor_tensor(out=dd[:], in0=lnp[:], in1=ln1[:],
                                        op=Alu.subtract)
                nc.vector.scalar_tensor_tensor(out=dd[:], in0=dd[:],
                                               scalar=0.0, in1=mskf,
                                               op0=Alu.bypass, op1=Alu.mult,
                                               accum_out=acc[:, 6 + s:7 + s])

            # ---------------- BCE tail over gathered [33,7] ----------------
            G8, valid = tail["G8"], tail["valid"]
            G = G8[:, 0:7]
            ln1b = sm.tile([33, 7], f32, tag="ln1b")
            l1s = sm.tile([33, 1], f32, tag="l1s")
            nc.scalar.activation(out=ln1b[:], in_=G, func=Act.Ln,
                                 bias=1.0, scale=-1.0, accum_out=l1s[:])
            lnpb = sm.tile([33, 7], f32, tag="lnpb")
            nc.scalar.activation(out=lnpb[:], in_=G, func=Act.Ln)
            ddb = sm.tile([33, 7], f32, tag="ddb")
            nc.vector.tensor_tensor(out=ddb[:], in0=lnpb[:], in1=ln1b[:],
                                    op=Alu.subtract)
            scr7 = sm.tile([33, 7], f32, tag="scr7")
            wsum = sm.tile([33, 1], f32, tag="wsum")
            nc.vector.scalar_tensor_tensor(out=scr7[:], in0=ddb[:],
                                           scalar=0.0, in1=lab33,
                                           op0=Alu.bypass, op1=Alu.mult,
                                           accum_out=wsum[:])
            tsum = sm.tile([33, 1], f32, tag="tsum")
            nc.vector.tensor_tensor(out=tsum[:], in0=wsum[:], in1=l1s[:],
                                    op=Alu.add)
            # tv per stack lands on its own 11 partition rows -> 3 acc columns
            tv = sm.tile([33, 1], f32, tag="tv")
            nc.vector.tensor_tensor(out=tv[:], in0=tsum[:], in1=valid[:],
                                    op=Alu.mult)
            for s in range(S):
                nc.vector.scalar_tensor_tensor(
                    out=acc[0:33, 9 + s:10 + s], in0=tv[:], scalar=0.0,
                    in1=ck[0:33, _C_SMASK + s:_C_SMASK + s + 1],
                    op0=Alu.bypass, op1=Alu.mult)

            # ---------------- final reduction ----------------
            acc2 = accp.tile([128, NACC], f32)
            nc.vector.tensor_copy(acc2[:], acc[:])
            ps1 = ps.tile([NACC, 1], f32, tag="ps1")
            nc.tensor.matmul(out=ps1[:], lhsT=acc2[:], rhs=ones[:], start=True, stop=True)
            s1 = sm.tile([NACC, 1], f32, tag="s1")
            nc.vector.tensor_copy(s1[:], ps1[:])
            ps2 = ps.tile([1, 9], f32, tag="ps2")
            nc.tensor.matmul(out=ps2[:], lhsT=s1[:], rhs=Wm, start=True, stop=True)
            res = sm.tile([1, 16], f32, tag="res")
            nc.vector.memset(res[:], 0.0)
            nc.vector.tensor_copy(res[0:1, 0:9], ps2[:])
            nc.sync.dma_start(out=out[:], in_=res[:])

    nc.finalize()
    return nc


def get_nc():
    if "nc" not in _CACHE:
        _CACHE["nc"] = _build_nc()
    return _CACHE["nc"]


def _make_wm():
    wm = np.zeros((NACC, 9), dtype=np.float32)
    for s in range(S):
        wm[s, s] = 1.0 / 11.0                # heat: accum is sum over K,pix
        wm[3 + s, 3 + s] = -1.0 / 65536.0    # mask: -(A+B)/HW
        wm[6 + s, 3 + s] = -1.0 / 65536.0
        wm[9 + s, 6 + s] = -1.0 / 77.0       # label: -sum/(7*11)
    return wm


def _consts():
    if "consts" in _CACHE:
        return _CACHE["consts"]
    rows = np.arange(33)
    s_idx = rows // 11
    k_idx = rows % 11
    ck = np.zeros((128, _CW), dtype=np.float32)
    ck[:, _C_IDENT:_C_IDENT + 128] = np.eye(128, dtype=np.float32)
    ck[0:33, _C_IOTP:_C_IOTP + 128] = 128.0 - np.arange(128, dtype=np.float32)
    ck[0:33, _C_IOTF:_C_IOTF + F] = float(F) - np.arange(F, dtype=np.float32)
    ck[0:33, _C_C1] = RG * P * (s_idx + 1.0) + k_idx
    ck[0:33, _C_C2] = P * F * (s_idx + 1.0) + F
    ck[0:NACC, _C_WM:_C_WM + 9] = _make_wm()
    ck[0:33, _C_SMASK:_C_SMASK + 3] = (s_idx[:, None] == np.arange(3)[None, :])
    _CACHE["consts"] = ck
    return ck


def make_in_maps(combined_preds, heatmaps, labels, masks):
    import ml_dtypes
    bf16 = ml_dtypes.bfloat16
    cpn = np.asarray(combined_preds, dtype=np.float32)
    hmn = np.asarray(heatmaps, dtype=np.float32)
    lbn = np.asarray(labels, dtype=np.float32)
    mkn = np.asarray(masks, dtype=np.float32)
    ck0 = _consts()
    in_maps = []
    for b in range(B):
        gt_sb = hmn[:, b].reshape(S, K, P, F)
        msk_sb = mkn[:, b, 0].reshape(S, 1, P, F)
        gtm = np.ascontiguousarray(
            np.concatenate([gt_sb, msk_sb], axis=1).transpose(0, 2, 1, 3)
        ).reshape(S, P, RG * F).astype(bf16)
        prarr = np.ascontiguousarray(
            cpn[:, b, K:2 * K].reshape(S, K, P, F).transpose(0, 2, 1, 3)
        ).reshape(S, P, K * F).astype(bf16)
        mp32 = np.ascontiguousarray(cpn[:, b, 2 * K]).reshape(S, P, F)
        lab8 = np.zeros((S * P * F, 8), dtype=np.float32)
        lab8[:, :7] = np.ascontiguousarray(
            cpn[:, b, 0:7].reshape(S, 7, P * F).transpose(0, 2, 1)
        ).reshape(S * P * F, 7)
        ck = ck0.copy()
        ck[0:33, _C_LAB:_C_LAB + 7] = np.tile(lbn[b], (3, 1))
        in_maps.append({
            "gtm": gtm,
            "pr": prarr,
            "mp": mp32,
            "lab8": lab8,
            "cpk": ck,
        })
    return in_maps


def run_spmd(in_maps, trace=False, **kw):
    from concourse.bass_utils import run_bass_kernel_spmd
    return run_bass_kernel_spmd(get_nc(), in_maps, core_ids=list(range(B)),
                                trace=trace, **kw)


def kernel(combined_preds, heatmaps, labels, masks):
    res = run_spmd(make_in_maps(combined_preds, heatmaps, labels, masks)).results
    heat = np.stack([res[b]["out"][0, 0:3] for b in range(B)]).astype(np.float32)
    mask_l = np.stack([res[b]["out"][0, 3:6] for b in range(B)]).astype(np.float32)
    label = np.stack([res[b]["out"][0, 6:9] for b in range(B)]).astype(np.float32)
    return (heat, label, mask_l)


# revision 22
# speedup vs baseline: 1.0504x; 1.0504x over previous
"""Trainium2 Bass kernel for nn_KeypointLoss (S=3, B=8, K=11, C=23, H=W=256).

Data-parallel over batch B=8 across 8 NeuronCores: core b computes the three
losses (heatmap / label / mask) for batch element b; host assembles [B,S].

v8: memory-regime optimized.
 - Bulk tensors (hm_pred, gt heatmaps, masks) staged in DRAM as bf16 (host
   cast): halves HBM traffic vs fp32 and enables DVE 2x tensor_tensor mode.
 - msk_pred stays fp32 (BCE log terms near p->1 are precision-critical).
 - Heat loss: DVE mult (+mask broadcast), DVE sub, ACT Square+accum.
 - Rowmax: DVE TT-max halving tree (2x bf16) + final 1x reduce on [P,K,64].
 - Label loss batched across all 3 stacks ([33 = S*K] partition rows):
   argmax via reversed-iota max trick (picks first match, always in range),
   peak row refetch + 7-channel label-pred gather via two indirect DMAs from
   row-contiguous DRAM tables, BCE on [33,7] fp32. is_equal*iota pairs are
   fused into single scalar_tensor_tensor ops.
 - Ln ACT table set pre-warmed during the DMA ramp.
"""

import numpy as np

S = 3
B = 8
K = 11
C = 23
P = 128
F = 512  # 256*256 = 128*512 plane layout
RG = K + 1  # gt+mask chunks per partition row in gtm
NACC = 12  # 3 heat + 3 ln1mp + 3 m*dd + 3 label cols

# column offsets inside the packed const param
_C_IDENT = 0          # [128, 128]
_C_IOTP = 128         # [33, 128] value 128-p
_C_IOTF = 256         # [33, 512] value 512-f
_C_C1 = 768           # [33, 1]  RG*128*(s+1) + k
_C_C2 = 769           # [33, 1]  65536*(s+1) + 512
_C_LAB = 770          # [33, 8]  labels tiled (7 used)
_C_WM = 778           # [12, 9]
_C_SMASK = 787        # [33, 3] stack row masks
_CW = 800

_CACHE = {}


def _build_nc():
    import concourse.bass as bass
    import concourse.bacc as bacc
    import concourse.mybir as mybir
    import concourse.tile as tile

    dt = mybir.dt
    f32, i32, bf16 = dt.float32, dt.int32, dt.bfloat16
    Alu = mybir.AluOpType
    Act = mybir.ActivationFunctionType
    AX = mybir.AxisListType.X

    nc = bacc.Bacc("TRN2", target_bir_lowering=False, debug=False)
    gtm = nc.declare_dram_parameter("gtm", [S, P, RG * F], bf16, isOutput=False)
    pr = nc.declare_dram_parameter("pr", [S, P, K * F], bf16, isOutput=False)
    mp = nc.declare_dram_parameter("mp", [S, P, F], f32, isOutput=False)
    lab8 = nc.declare_dram_parameter("lab8", [S * P * F, 8], f32, isOutput=False)
    ckp = nc.declare_dram_parameter("cpk", [128, _CW], f32, isOutput=False)
    out = nc.declare_dram_parameter("out", [1, 16], f32, isOutput=True)

    # row-contiguous table for the gt-row refetch: row (s,p,r), r in [0,12)
    gtrows = gtm[:].rearrange("s p (r f) -> (s p r) f", f=F)

    with tile.TileContext(nc) as tc:
        with (
            tc.tile_pool(name="const", bufs=1) as cst,
            tc.tile_pool(name="accp", bufs=1) as accp,
            tc.tile_pool(name="gtp", bufs=2) as gtp,
            tc.tile_pool(name="prp", bufs=2) as prp,
            tc.tile_pool(name="mpp", bufs=2) as mpp,
            tc.tile_pool(name="rmx", bufs=2) as rmx,
            tc.tile_pool(name="lns", bufs=2) as lns,
            tc.tile_pool(name="sm", bufs=2) as sm,
            tc.tile_pool(name="ps", bufs=2, space="PSUM") as ps,
        ):
            # ---------------- constants: one DMA ----------------
            ck = cst.tile([128, _CW], f32)
            nc.sync.dma_start(out=ck[:], in_=ckp[:])
            ident = ck[:, _C_IDENT:_C_IDENT + 128]
            iotp_t = ck[0:33, _C_IOTP:_C_IOTP + 128]
            iotf_t = ck[0:33, _C_IOTF:_C_IOTF + F]
            c1t = ck[0:33, _C_C1:_C_C1 + 1]
            c2t = ck[0:33, _C_C2:_C_C2 + 1]
            lab33 = ck[0:33, _C_LAB:_C_LAB + 7]
            Wm = ck[0:NACC, _C_WM:_C_WM + 9]
            ones = cst.tile([128, 1], f32)
            nc.vector.memset(ones[:], 1.0)

            acc = accp.tile([128, NACC], f32)
            nc.vector.memset(acc[:], 0.0)
            rmall = accp.tile([128, 33], f32)

            # warm the Ln/Square ACT table set during the DMA ramp
            lnwarm = accp.tile([1, 1], f32)
            nc.scalar.activation(out=lnwarm[:], in_=ones[0:1, 0:1], func=Act.Ln)

            # ---------------- per-stack main loop ----------------
            for s in range(S):
                gtt = gtp.tile([P, RG, F], bf16, tag="gt")
                nc.sync.dma_start(out=gtt[:], in_=gtm[s])
                prt = prp.tile([P, K, F], bf16, tag="pr")
                nc.scalar.dma_start(out=prt[:], in_=pr[s])
                mpt = mpp.tile([P, F], f32, tag="mp")
                nc.scalar.dma_start(out=mpt[:], in_=mp[s])

                gt = gtt[:, 0:K, :]
                mskb = gtt[:, K:RG, :]   # [P,1,F]
                mskf = gtt[:, K, :]      # [P,F]

                # ---- rowmax tree: label pipeline depends only on gt
                t256 = rmx.tile([P, K, 256], bf16, tag="t256")
                nc.vector.tensor_tensor(out=t256[:], in0=gt[:, :, 0:256],
                                        in1=gt[:, :, 256:512], op=Alu.max)
                t128 = rmx.tile([P, K, 128], bf16, tag="t128")
                nc.vector.tensor_tensor(out=t128[:], in0=t256[:, :, 0:128],
                                        in1=t256[:, :, 128:256], op=Alu.max)
                t64 = rmx.tile([P, K, 64], bf16, tag="t64")
                nc.vector.tensor_tensor(out=t64[:], in0=t128[:, :, 0:64],
                                        in1=t128[:, :, 64:128], op=Alu.max)
                nc.vector.tensor_reduce(out=rmall[:, s * K:(s + 1) * K],
                                        in_=t64[:], axis=AX, op=Alu.max)

                # ---- heatmap loss: sum_{k,pix} (pred*mask - gt)^2
                nc.vector.tensor_tensor(out=prt[:], in0=prt[:],
                                        in1=mskb.to_broadcast([P, K, F]),
                                        op=Alu.mult)
                nc.vector.tensor_tensor(out=prt[:], in0=prt[:], in1=gt,
                                        op=Alu.subtract)
                nc.scalar.activation(out=prt[:], in_=prt[:], func=Act.Square,
                                     accum_out=acc[:, s:s + 1])

                # ---- mask loss: BCE(msk_pred, mask) summed
                ln1 = lns.tile([P, F], bf16, tag="ln1")
                lnp = lns.tile([P, F], bf16, tag="lnp")
                nc.scalar.activation(out=ln1[:], in_=mpt[:], func=Act.Ln,
                                     bias=1.0, scale=-1.0,
                                     accum_out=acc[:, 3 + s:4 + s])
                nc.scalar.activation(out=lnp[:], in_=mpt[:], func=Act.Ln)
                dd = lns.tile([P, F], bf16, tag="dd")
                nc.vector.tensor_tensor(out=dd[:], in0=lnp[:], in1=ln1[:],
                                        op=Alu.subtract)
                nc.vector.scalar_tensor_tensor(out=dd[:], in0=dd[:],
                                               scalar=0.0, in1=mskf,
                                               op0=Alu.bypass, op1=Alu.mult,
                                               accum_out=acc[:, 6 + s:7 + s])

            # ---------------- batched label loss tail ([33] rows) ----------------
            pt = ps.tile([33, 128], f32, tag="pt")
            nc.tensor.transpose(out=pt[:], in_=rmall[:], identity=ident)
            Mx = sm.tile([33, 1], f32, tag="Mx")
            nc.vector.tensor_reduce(out=Mx[:], in_=pt[:], axis=AX, op=Alu.max)
            # fused: scrp = (pt == Mx) * iotp  -> max -> 128 - p*
            scrp = sm.tile([33, 128], f32, tag="scrp")
            nc.vector.scalar_tensor_tensor(out=scrp[:], in0=pt[:],
                                           scalar=Mx[:, 0:1], in1=iotp_t,
                                           op0=Alu.is_equal, op1=Alu.mult)
            pmr = sm.tile([33, 1], f32, tag="pmr")  # 128 - p*
            nc.vector.tensor_reduce(out=pmr[:], in_=scrp[:], axis=AX, op=Alu.max)

            # refetch winning gt row: gtrows index = c1 - 12*pmr
            idxg = sm.tile([33, 1], f32, tag="idxg")
            nc.vector.scalar_tensor_tensor(out=idxg[:], in0=pmr[:],
                                           scalar=-float(RG), in1=c1t,
                                           op0=Alu.mult, op1=Alu.add)
            idxgi = sm.tile([33, 1], i32, tag="idxgi")
            nc.vector.tensor_copy(idxgi[:], idxg[:])
            grow = sm.tile([33, F], bf16, tag="grow")
            nc.gpsimd.indirect_dma_start(
                out=grow[:], out_offset=None, in_=gtrows,
                in_offset=bass.IndirectOffsetOnAxis(ap=idxgi[:, 0:1], axis=0))
            # fused: scrf = (grow == Mx) * iotf -> max -> 512 - f*
            scrf = sm.tile([33, F], f32, tag="scrf")
            nc.vector.scalar_tensor_tensor(out=scrf[:], in0=grow[:],
                                           scalar=Mx[:, 0:1], in1=iotf_t,
                                           op0=Alu.is_equal, op1=Alu.mult)
            fmr = sm.tile([33, 1], f32, tag="fmr")  # 512 - f*
            nc.vector.tensor_reduce(out=fmr[:], in_=scrf[:], axis=AX, op=Alu.max)

            # label-pred gather row: lab8 index = c2 - 512*pmr - fmr
            t1 = sm.tile([33, 1], f32, tag="t1")
            nc.vector.scalar_tensor_tensor(out=t1[:], in0=pmr[:],
                                           scalar=-512.0, in1=c2t,
                                           op0=Alu.mult, op1=Alu.add)
            gx = sm.tile([33, 1], f32, tag="gx")
            nc.vector.tensor_tensor(out=gx[:], in0=t1[:], in1=fmr[:],
                                    op=Alu.subtract)
            gxi = sm.tile([33, 1], i32, tag="gxi")
            nc.vector.tensor_copy(gxi[:], gx[:])
            G8 = sm.tile([33, 8], f32, tag="G8")
            nc.gpsimd.indirect_dma_start(
                out=G8[:], out_offset=None, in_=lab8[:],
                in_offset=bass.IndirectOffsetOnAxis(ap=gxi[:, 0:1], axis=0))

            valid = sm.tile([33, 1], f32, tag="valid")
            nc.vector.tensor_scalar(out=valid[:], in0=Mx[:], scalar1=1.0,
                                    scalar2=None, op0=Alu.is_equal)

            # BCE over gathered [33,7]
            G = G8[:, 0:7]
            ln1b = sm.tile([33, 7], f32, tag="ln1b")
            l1s = sm.tile([33, 1], f32, tag="l1s")
            nc.scalar.activation(out=ln1b[:], in_=G, func=Act.Ln,
                                 bias=1.0, scale=-1.0, accum_out=l1s[:])
            lnpb = sm.tile([33, 7], f32, tag="lnpb")
            nc.scalar.activation(out=lnpb[:], in_=G, func=Act.Ln)
            ddb = sm.tile([33, 7], f32, tag="ddb")
            nc.vector.tensor_tensor(out=ddb[:], in0=lnpb[:], in1=ln1b[:],
                                    op=Alu.subtract)
            scr7 = sm.tile([33, 7], f32, tag="scr7")
            wsum = sm.tile([33, 1], f32, tag="wsum")
            nc.vector.scalar_tensor_tensor(out=scr7[:], in0=ddb[:],
                                           scalar=0.0, in1=lab33,
                                           op0=Alu.bypass, op1=Alu.mult,
                                           accum_out=wsum[:])
            # fused: tv = (wsum + l1s) * valid
            tv = sm.tile([33, 1], f32, tag="tv")
            nc.vector.scalar_tensor_tensor(out=tv[:], in0=wsum[:],
                                           scalar=l1s[:, 0:1], in1=valid[:],
                                           op0=Alu.add, op1=Alu.mult)
            # one op writes all three per-stack label columns via smask
            nc.vector.scalar_tensor_tensor(
                out=acc[0:33, 9:12], in0=tv[:, 0:1].to_broadcast([33, 3]),
                scalar=0.0, in1=ck[0:33, _C_SMASK:_C_SMASK + 3],
                op0=Alu.bypass, op1=Alu.mult)

            # ---------------- final reduction ----------------
            acc2 = accp.tile([128, NACC], f32)
            nc.vector.tensor_copy(acc2[:], acc[:])
            ps1 = ps.tile([NACC, 1], f32, tag="ps1")
            nc.tensor.matmul(out=ps1[:], lhsT=acc2[:], rhs=ones[:], start=True, stop=True)
            s1 = sm.tile([NACC, 1], f32, tag="s1")
            nc.vector.tensor_copy(s1[:], ps1[:])
            ps2 = ps.tile([1, 9], f32, tag="ps2")
            nc.tensor.matmul(out=ps2[:], lhsT=s1[:], rhs=Wm, start=True, stop=True)
            res = sm.tile([1, 16], f32, tag="res")
            nc.vector.memset(res[:], 0.0)
            nc.vector.tensor_copy(res[0:1, 0:9], ps2[:])
            nc.sync.dma_start(out=out[:], in_=res[:])

    nc.finalize()
    return nc


def get_nc():
    if "nc" not in _CACHE:
        _CACHE["nc"] = _build_nc()
    return _CACHE["nc"]


def _make_wm():
    wm = np.zeros((NACC, 9), dtype=np.float32)
    for s in range(S):
        wm[s, s] = 1.0 / 11.0                # heat: accum is sum over K,pix
        wm[3 + s, 3 + s] = -1.0 / 65536.0    # mask: -(A+B)/HW
        wm[6 + s, 3 + s] = -1.0 / 65536.0
        wm[9 + s, 6 + s] = -1.0 / 77.0       # label: -sum/(7*11)
    return wm


def _consts():
    if "consts" in _CACHE:
        return _CACHE["consts"]
    rows = np.arange(33)
    s_idx = rows // 11
    k_idx = rows % 11
    ck = np.zeros((128, _CW), dtype=np.float32)
    ck[:, _C_IDENT:_C_IDENT + 128] = np.eye(128, dtype=np.float32)
    ck[0:33, _C_IOTP:_C_IOTP + 128] = 128.0 - np.arange(128, dtype=np.float32)
    ck[0:33, _C_IOTF:_C_IOTF + F] = float(F) - np.arange(F, dtype=np.float32)
    ck[0:33, _C_C1] = RG * P * (s_idx + 1.0) + k_idx
    ck[0:33, _C_C2] = P * F * (s_idx + 1.0) + F
    ck[0:NACC, _C_WM:_C_WM + 9] = _make_wm()
    ck[0:33, _C_SMASK:_C_SMASK + 3] = (s_idx[:, None] == np.arange(3)[None, :])
    _CACHE["consts"] = ck
    return ck


def make_in_maps(combined_preds, heatmaps, labels, masks):
    import ml_dtypes
    bf16 = ml_dtypes.bfloat16
    cpn = np.asarray(combined_preds, dtype=np.float32)
    hmn = np.asarray(heatmaps, dtype=np.float32)
    lbn = np.asarray(labels, dtype=np.float32)
    mkn = np.asarray(masks, dtype=np.float32)
    ck0 = _consts()
    in_maps = []
    for b in range(B):
        gt_sb = hmn[:, b].reshape(S, K, P, F)
        msk_sb = mkn[:, b, 0].reshape(S, 1, P, F)
        gtm = np.ascontiguousarray(
            np.concatenate([gt_sb, msk_sb], axis=1).transpose(0, 2, 1, 3)
        ).reshape(S, P, RG * F).astype(bf16)
        prarr = np.ascontiguousarray(
            cpn[:, b, K:2 * K].reshape(S, K, P, F).transpose(0, 2, 1, 3)
        ).reshape(S, P, K * F).astype(bf16)
        mp32 = np.ascontiguousarray(cpn[:, b, 2 * K]).reshape(S, P, F)
        lab8 = np.zeros((S * P * F, 8), dtype=np.float32)
        lab8[:, :7] = np.ascontiguousarray(
            cpn[:, b, 0:7].reshape(S, 7, P * F).transpose(0, 2, 1)
        ).reshape(S * P * F, 7)
        ck = ck0.copy()
        ck[0:33, _C_LAB:_C_LAB + 7] = np.tile(lbn[b], (3, 1))
        in_maps.append({
            "gtm": gtm,
            "pr": prarr,
            "mp": mp32,
            "lab8": lab8,
            "cpk": ck,
        })
    return in_maps


def run_spmd(in_maps, trace=False, **kw):
    from concourse.bass_utils import run_bass_kernel_spmd
    return run_bass_kernel_spmd(get_nc(), in_maps, core_ids=list(range(B)),
                                trace=trace, **kw)


def kernel(combined_preds, heatmaps, labels, masks):
    res = run_spmd(make_in_maps(combined_preds, heatmaps, labels, masks)).results
    heat = np.stack([res[b]["out"][0, 0:3] for b in range(B)]).astype(np.float32)
    mask_l = np.stack([res[b]["out"][0, 3:6] for b in range(B)]).astype(np.float32)
    label = np.stack([res[b]["out"][0, 6:9] for b in range(B)]).astype(np.float32)
    return (heat, label, mask_l)


# revision 25
# speedup vs baseline: 1.0974x; 1.0447x over previous
"""Trainium2 Bass kernel for nn_KeypointLoss (S=3, B=8, K=11, C=23, H=W=256).

Data-parallel over batch B=8 across 8 NeuronCores: core b computes the three
losses (heatmap / label / mask) for batch element b; host assembles [B,S].

v8: memory-regime optimized.
 - Bulk tensors (hm_pred, gt heatmaps, masks) staged in DRAM as bf16 (host
   cast): halves HBM traffic vs fp32 and enables DVE 2x tensor_tensor mode.
 - msk_pred stays fp32 (BCE log terms near p->1 are precision-critical).
 - Heat loss: DVE mult (+mask broadcast), DVE sub, ACT Square+accum.
 - Rowmax: DVE TT-max halving tree (2x bf16) + final 1x reduce on [P,K,64].
 - Label loss batched across all 3 stacks ([33 = S*K] partition rows):
   argmax via reversed-iota max trick (picks first match, always in range),
   peak row refetch + 7-channel label-pred gather via two indirect DMAs from
   row-contiguous DRAM tables, BCE on [33,7] fp32. is_equal*iota pairs are
   fused into single scalar_tensor_tensor ops.
 - Ln ACT table set pre-warmed during the DMA ramp.
"""

import numpy as np

S = 3
B = 8
K = 11
C = 23
P = 128
F = 512  # 256*256 = 128*512 plane layout
RG = K + 1  # gt+mask chunks per partition row in gtm
NACC = 12  # 3 heat + 3 ln1mp + 3 m*dd + 3 label cols

# column offsets inside the packed const param
_C_IDENT = 0          # [128, 128]
_C_IOTP = 128         # [33, 128] value 128-p
_C_IOTF = 256         # [33, 512] value 512-f
_C_C1 = 768           # [33, 1]  RG*128*(s+1) + k
_C_C2 = 769           # [33, 1]  65536*(s+1) + 512
_C_LAB = 770          # [33, 8]  labels tiled (7 used)
_C_WM = 778           # [12, 9]
_C_SMASK = 787        # [33, 3] stack row masks
_CW = 800

_CACHE = {}


def _build_nc():
    import concourse.bass as bass
    import concourse.bacc as bacc
    import concourse.mybir as mybir
    import concourse.tile as tile

    dt = mybir.dt
    f32, i32, bf16 = dt.float32, dt.int32, dt.bfloat16
    Alu = mybir.AluOpType
    Act = mybir.ActivationFunctionType
    AX = mybir.AxisListType.X

    nc = bacc.Bacc("TRN2", target_bir_lowering=False, debug=False)
    gtm = nc.declare_dram_parameter("gtm", [S, P, RG * F], bf16, isOutput=False)
    pr = nc.declare_dram_parameter("pr", [S, P, K * F], bf16, isOutput=False)
    mp = nc.declare_dram_parameter("mp", [S, P, F], f32, isOutput=False)
    lab8 = nc.declare_dram_parameter("lab8", [S * P * F, 8], f32, isOutput=False)
    ckp = nc.declare_dram_parameter("cpk", [128, _CW], f32, isOutput=False)
    out = nc.declare_dram_parameter("out", [1, 16], f32, isOutput=True)

    # row-contiguous table for the gt-row refetch: row (s,p,r), r in [0,12)
    gtrows = gtm[:].rearrange("s p (r f) -> (s p r) f", f=F)

    with tile.TileContext(nc) as tc:
        with (
            tc.tile_pool(name="const", bufs=1) as cst,
            tc.tile_pool(name="accp", bufs=1) as accp,
            tc.tile_pool(name="gtp", bufs=3) as gtp,
            tc.tile_pool(name="prp", bufs=2) as prp,
            tc.tile_pool(name="mpp", bufs=2) as mpp,
            tc.tile_pool(name="rmx", bufs=2) as rmx,
            tc.tile_pool(name="lns", bufs=2) as lns,
            tc.tile_pool(name="sm", bufs=2) as sm,
            tc.tile_pool(name="ps", bufs=2, space="PSUM") as ps,
        ):
            # ---------------- constants: one DMA ----------------
            # issued on the scalar ring AFTER the bulk loads (consts are only
            # needed by the label tail; keeps the first gt load at ring head)
            ck = cst.tile([128, _CW], f32)
            ident = ck[:, _C_IDENT:_C_IDENT + 128]
            iotp_t = ck[0:33, _C_IOTP:_C_IOTP + 128]
            iotf_t = ck[0:33, _C_IOTF:_C_IOTF + F]
            c1t = ck[0:33, _C_C1:_C_C1 + 1]
            c2t = ck[0:33, _C_C2:_C_C2 + 1]
            lab33 = ck[0:33, _C_LAB:_C_LAB + 7]
            Wm = ck[0:NACC, _C_WM:_C_WM + 9]
            ones = cst.tile([128, 1], f32)
            nc.vector.memset(ones[:], 1.0)

            acc = accp.tile([128, NACC], f32)
            nc.vector.memset(acc[:], 0.0)
            rmall = accp.tile([128, 33], f32)

            # warm the Ln/Square ACT table set during the DMA ramp
            lnwarm = accp.tile([1, 1], f32)
            nc.scalar.activation(out=lnwarm[:], in_=ones[0:1, 0:1], func=Act.Ln)

            # ---------------- per-stack main loop ----------------
            for s in range(S):
                gtt = gtp.tile([P, RG, F], bf16, tag="gt")
                if s == 0:
                    # first load feeds the first compute: use both HWDGE rings
                    nc.sync.dma_start(out=gtt[:, 0:6, :], in_=gtm[s][:, 0:6 * F])
                    nc.scalar.dma_start(out=gtt[:, 6:RG, :],
                                        in_=gtm[s][:, 6 * F:RG * F])
                else:
                    nc.sync.dma_start(out=gtt[:], in_=gtm[s])
                prt = prp.tile([P, K, F], bf16, tag="pr")
                nc.scalar.dma_start(out=prt[:], in_=pr[s])
                mpt = mpp.tile([P, F], f32, tag="mp")
                nc.scalar.dma_start(out=mpt[:], in_=mp[s])
                if s == S - 1:
                    nc.scalar.dma_start(out=ck[:], in_=ckp[:])

                gt = gtt[:, 0:K, :]
                mskb = gtt[:, K:RG, :]   # [P,1,F]
                mskf = gtt[:, K, :]      # [P,F]

                # ---- rowmax tree: label pipeline depends only on gt
                t256 = rmx.tile([P, K, 256], bf16, tag="t256")
                nc.vector.tensor_tensor(out=t256[:], in0=gt[:, :, 0:256],
                                        in1=gt[:, :, 256:512], op=Alu.max)
                t128 = rmx.tile([P, K, 128], bf16, tag="t128")
                nc.vector.tensor_tensor(out=t128[:], in0=t256[:, :, 0:128],
                                        in1=t256[:, :, 128:256], op=Alu.max)
                t64 = rmx.tile([P, K, 64], bf16, tag="t64")
                nc.vector.tensor_tensor(out=t64[:], in0=t128[:, :, 0:64],
                                        in1=t128[:, :, 64:128], op=Alu.max)
                nc.vector.tensor_reduce(out=rmall[:, s * K:(s + 1) * K],
                                        in_=t64[:], axis=AX, op=Alu.max)

                # ---- heatmap loss: sum_{k,pix} (pred*mask - gt)^2
                nc.vector.tensor_tensor(out=prt[:], in0=prt[:],
                                        in1=mskb.to_broadcast([P, K, F]),
                                        op=Alu.mult)
                nc.vector.tensor_tensor(out=prt[:], in0=prt[:], in1=gt,
                                        op=Alu.subtract)
                nc.scalar.activation(out=prt[:], in_=prt[:], func=Act.Square,
                                     accum_out=acc[:, s:s + 1])

                # ---- mask loss: BCE(msk_pred, mask) summed
                ln1 = lns.tile([P, F], bf16, tag="ln1")
                lnp = lns.tile([P, F], bf16, tag="lnp")
                nc.scalar.activation(out=ln1[:], in_=mpt[:], func=Act.Ln,
                                     bias=1.0, scale=-1.0,
                                     accum_out=acc[:, 3 + s:4 + s])
                nc.scalar.activation(out=lnp[:], in_=mpt[:], func=Act.Ln)
                dd = lns.tile([P, F], bf16, tag="dd")
                nc.vector.tensor_tensor(out=dd[:], in0=lnp[:], in1=ln1[:],
                                        op=Alu.subtract)
                nc.vector.scalar_tensor_tensor(out=dd[:], in0=dd[:],
                                               scalar=0.0, in1=mskf,
                                               op0=Alu.bypass, op1=Alu.mult,
                                               accum_out=acc[:, 6 + s:7 + s])

            # ---------------- batched label loss tail ([33] rows) ----------------
            pt = ps.tile([33, 128], f32, tag="pt")
            nc.tensor.transpose(out=pt[:], in_=rmall[:], identity=ident)
            Mx = sm.tile([33, 1], f32, tag="Mx")
            nc.vector.tensor_reduce(out=Mx[:], in_=pt[:], axis=AX, op=Alu.max)
            # fused: scrp = (pt == Mx) * iotp  -> max -> 128 - p*
            scrp = sm.tile([33, 128], f32, tag="scrp")
            nc.vector.scalar_tensor_tensor(out=scrp[:], in0=pt[:],
                                           scalar=Mx[:, 0:1], in1=iotp_t,
                                           op0=Alu.is_equal, op1=Alu.mult)
            pmr = sm.tile([33, 1], f32, tag="pmr")  # 128 - p*
            nc.vector.tensor_reduce(out=pmr[:], in_=scrp[:], axis=AX, op=Alu.max)

            # refetch winning gt row: gtrows index = c1 - 12*pmr
            idxg = sm.tile([33, 1], f32, tag="idxg")
            nc.vector.scalar_tensor_tensor(out=idxg[:], in0=pmr[:],
                                           scalar=-float(RG), in1=c1t,
                                           op0=Alu.mult, op1=Alu.add)
            idxgi = sm.tile([33, 1], i32, tag="idxgi")
            nc.vector.tensor_copy(idxgi[:], idxg[:])
            grow = sm.tile([33, F], bf16, tag="grow")
            nc.gpsimd.indirect_dma_start(
                out=grow[:], out_offset=None, in_=gtrows,
                in_offset=bass.IndirectOffsetOnAxis(ap=idxgi[:, 0:1], axis=0))
            # fused: scrf = (grow == Mx) * iotf -> max -> 512 - f*
            scrf = sm.tile([33, F], f32, tag="scrf")
            nc.vector.scalar_tensor_tensor(out=scrf[:], in0=grow[:],
                                           scalar=Mx[:, 0:1], in1=iotf_t,
                                           op0=Alu.is_equal, op1=Alu.mult)
            fmr = sm.tile([33, 1], f32, tag="fmr")  # 512 - f*
            nc.vector.tensor_reduce(out=fmr[:], in_=scrf[:], axis=AX, op=Alu.max)

            # label-pred gather row: lab8 index = c2 - 512*pmr - fmr
            t1 = sm.tile([33, 1], f32, tag="t1")
            nc.vector.scalar_tensor_tensor(out=t1[:], in0=pmr[:],
                                           scalar=-512.0, in1=c2t,
                                           op0=Alu.mult, op1=Alu.add)
            gx = sm.tile([33, 1], f32, tag="gx")
            nc.vector.tensor_tensor(out=gx[:], in0=t1[:], in1=fmr[:],
                                    op=Alu.subtract)
            gxi = sm.tile([33, 1], i32, tag="gxi")
            nc.vector.tensor_copy(gxi[:], gx[:])
            G8 = sm.tile([33, 8], f32, tag="G8")
            nc.gpsimd.indirect_dma_start(
                out=G8[:], out_offset=None, in_=lab8[:],
                in_offset=bass.IndirectOffsetOnAxis(ap=gxi[:, 0:1], axis=0))

            valid = sm.tile([33, 1], f32, tag="valid")
            nc.vector.tensor_scalar(out=valid[:], in0=Mx[:], scalar1=1.0,
                                    scalar2=None, op0=Alu.is_equal)

            # BCE over gathered [33,7]
            G = G8[:, 0:7]
            ln1b = sm.tile([33, 7], f32, tag="ln1b")
            l1s = sm.tile([33, 1], f32, tag="l1s")
            nc.scalar.activation(out=ln1b[:], in_=G, func=Act.Ln,
                                 bias=1.0, scale=-1.0, accum_out=l1s[:])
            lnpb = sm.tile([33, 7], f32, tag="lnpb")
            nc.scalar.activation(out=lnpb[:], in_=G, func=Act.Ln)
            ddb = sm.tile([33, 7], f32, tag="ddb")
            nc.vector.tensor_tensor(out=ddb[:], in0=lnpb[:], in1=ln1b[:],
                                    op=Alu.subtract)
            scr7 = sm.tile([33, 7], f32, tag="scr7")
            wsum = sm.tile([33, 1], f32, tag="wsum")
            nc.vector.scalar_tensor_tensor(out=scr7[:], in0=ddb[:],
                                           scalar=0.0, in1=lab33,
                                           op0=Alu.bypass, op1=Alu.mult,
                                           accum_out=wsum[:])
            # fused: tv = (wsum + l1s) * valid
            tv = sm.tile([33, 1], f32, tag="tv")
            nc.vector.scalar_tensor_tensor(out=tv[:], in0=wsum[:],
                                           scalar=l1s[:, 0:1], in1=valid[:],
                                           op0=Alu.add, op1=Alu.mult)
            # one op writes all three per-stack label columns via smask
            nc.vector.scalar_tensor_tensor(
                out=acc[0:33, 9:12], in0=tv[:, 0:1].to_broadcast([33, 3]),
                scalar=0.0, in1=ck[0:33, _C_SMASK:_C_SMASK + 3],
                op0=Alu.bypass, op1=Alu.mult)

            # ---------------- final reduction ----------------
            acc2 = accp.tile([128, NACC], f32)
            nc.vector.tensor_copy(acc2[:], acc[:])
            ps1 = ps.tile([NACC, 1], f32, tag="ps1")
            nc.tensor.matmul(out=ps1[:], lhsT=acc2[:], rhs=ones[:], start=True, stop=True)
            s1 = sm.tile([NACC, 1], f32, tag="s1")
            nc.vector.tensor_copy(s1[:], ps1[:])
            ps2 = ps.tile([1, 9], f32, tag="ps2")
            nc.tensor.matmul(out=ps2[:], lhsT=s1[:], rhs=Wm, start=True, stop=True)
            res = sm.tile([1, 16], f32, tag="res")
            nc.vector.memset(res[:], 0.0)
            nc.vector.tensor_copy(res[0:1, 0:9], ps2[:])
            nc.sync.dma_start(out=out[:], in_=res[:])

    nc.finalize()
    return nc


def get_nc():
    if "nc" not in _CACHE:
        _CACHE["nc"] = _build_nc()
    return _CACHE["nc"]


def _make_wm():
    wm = np.zeros((NACC, 9), dtype=np.float32)
    for s in range(S):
        wm[s, s] = 1.0 / 11.0                # heat: accum is sum over K,pix
        wm[3 + s, 3 + s] = -1.0 / 65536.0    # mask: -(A+B)/HW
        wm[6 + s, 3 + s] = -1.0 / 65536.0
        wm[9 + s, 6 + s] = -1.0 / 77.0       # label: -sum/(7*11)
    return wm


def _consts():
    if "consts" in _CACHE:
        return _CACHE["consts"]
    rows = np.arange(33)
    s_idx = rows // 11
    k_idx = rows % 11
    ck = np.zeros((128, _CW), dtype=np.float32)
    ck[:, _C_IDENT:_C_IDENT + 128] = np.eye(128, dtype=np.float32)
    ck[0:33, _C_IOTP:_C_IOTP + 128] = 128.0 - np.arange(128, dtype=np.float32)
    ck[0:33, _C_IOTF:_C_IOTF + F] = float(F) - np.arange(F, dtype=np.float32)
    ck[0:33, _C_C1] = RG * P * (s_idx + 1.0) + k_idx
    ck[0:33, _C_C2] = P * F * (s_idx + 1.0) + F
    ck[0:NACC, _C_WM:_C_WM + 9] = _make_wm()
    ck[0:33, _C_SMASK:_C_SMASK + 3] = (s_idx[:, None] == np.arange(3)[None, :])
    _CACHE["consts"] = ck
    return ck


def make_in_maps(combined_preds, heatmaps, labels, masks):
    import ml_dtypes
    bf16 = ml_dtypes.bfloat16
    cpn = np.asarray(combined_preds, dtype=np.float32)
    hmn = np.asarray(heatmaps, dtype=np.float32)
    lbn = np.asarray(labels, dtype=np.float32)
    mkn = np.asarray(masks, dtype=np.float32)
    ck0 = _consts()
    in_maps = []
    for b in range(B):
        gt_sb = hmn[:, b].reshape(S, K, P, F)
        msk_sb = mkn[:, b, 0].reshape(S, 1, P, F)
        gtm = np.ascontiguousarray(
            np.concatenate([gt_sb, msk_sb], axis=1).transpose(0, 2, 1, 3)
        ).reshape(S, P, RG * F).astype(bf16)
        prarr = np.ascontiguousarray(
            cpn[:, b, K:2 * K].reshape(S, K, P, F).transpose(0, 2, 1, 3)
        ).reshape(S, P, K * F).astype(bf16)
        mp32 = np.ascontiguousarray(cpn[:, b, 2 * K]).reshape(S, P, F)
        lab8 = np.zeros((S * P * F, 8), dtype=np.float32)
        lab8[:, :7] = np.ascontiguousarray(
            cpn[:, b, 0:7].reshape(S, 7, P * F).transpose(0, 2, 1)
        ).reshape(S * P * F, 7)
        ck = ck0.copy()
        ck[0:33, _C_LAB:_C_LAB + 7] = np.tile(lbn[b], (3, 1))
        in_maps.append({
            "gtm": gtm,
            "pr": prarr,
            "mp": mp32,
            "lab8": lab8,
            "cpk": ck,
        })
    return in_maps


def run_spmd(in_maps, trace=False, **kw):
    from concourse.bass_utils import run_bass_kernel_spmd
    return run_bass_kernel_spmd(get_nc(), in_maps, core_ids=list(range(B)),
                                trace=trace, **kw)


def kernel(combined_preds, heatmaps, labels, masks):
    res = run_spmd(make_in_maps(combined_preds, heatmaps, labels, masks)).results
    heat = np.stack([res[b]["out"][0, 0:3] for b in range(B)]).astype(np.float32)
    mask_l = np.stack([res[b]["out"][0, 3:6] for b in range(B)]).astype(np.float32)
    label = np.stack([res[b]["out"][0, 6:9] for b in range(B)]).astype(np.float32)
    return (heat, label, mask_l)
